# revision 36
# baseline (speedup 1.0000x reference)
"""GAT (2-layer, PyG-style) on 8 Trainium2 NeuronCores.

Strategy (node/graph-parallel per the sharding hint):
  - Nodes partitioned into 8 contiguous ranges (6250/core); edges assigned to
    the core owning their DST node, processed per 128-dst window.
  - Node phase sharded: each core computes h_aug = x @ [W | W@att_src | W@att_dst]
    for its own nodes into 768-B table rows, then a chunked AllGather
    replicates the gather table [N, 384] bf16.
  - Edge phase per core: edges form two streams (A: table row < 25600,
    B: rest — keeps int16 gather indices in range), each a contiguous
    sequence of 128-edge tiles with window segments padded to the max count
    across cores so the program is SPMD-identical. Source rows are fetched
    with batched SWDGE dma_gather (1024 rows = 8 tiles per instruction, the
    ucode max), round-robin over 4 SWDGE queues, into a 32-tile SBUF ring
    per stream. Window boundaries fall mid-tile; boundary tiles are consumed
    by both windows with separate fp8 one-hot slices.
  - Attention: logits assembled on-chip; segment softmax + scatter-add done
    as TensorE matmuls against host-built fp8 one-hots, with exp folded into
    the matmul rhs and 1/sum applied per-dst at the end.
  - Global mean-pool via matmul with a host-built node->graph map, AllReduce
    of the [256, 50] partial, then the FC layer replicated.
"""

import os
import sys

sys.path.insert(0, "/opt/trn_rl_repo")

import numpy as np
import ml_dtypes

N_NODES, N_EDGES = 50000, 800000
IN_C, HID_C, OUT_C, HEADS = 256, 64, 256, 4
N_GRAPHS = 50
NEG_SLOPE = 0.2
NCORES = 8
WIN = 128         # dst nodes per aggregation window (psum partitions)
P = 128
TBW = 256         # gather-table row width in bf16 units: 512 B =
                  # [h fp8 (256 B) | a_src bf16 (2H B) | pad]
ACOL = 128        # bf16-unit column of a_src in a table row
PCOL = 256        # f32 column of a_src in the node-phase psum (264-col matmul)
NQ = 4            # SWDGE queues
GMAX = 8          # tiles per dma_gather (1024 idx = ucode max)
RINGL = {1: 72, 2: 32}  # ring tiles per stream, per layer (multiples of GMAX);
                        # separate rings let L2's gathers prefetch during L1's
                        # tail; L1 gets the depth, L2 the early start
PFL = {1: 56, 2: 16}    # prefetch depth (must stay ≥ max window span + 1
                        # below RING so slot reuse only hits windows whose
                        # reads are already emitted)

BF16 = ml_dtypes.bfloat16
FP8 = ml_dtypes.float8_e4m3

CHUNK_TILES = 25  # node-tiles per AllGather chunk
SPLIT = NCORES * CHUNK_TILES * P  # 25600: A/B gather split (chunk-0 boundary)

LAST_EXEC_NS = None  # set by kernel() when GAT_TRACE=1


# --------------------------------------------------------------------------
# host-side preprocessing
# --------------------------------------------------------------------------

def balance_nodes(dst, n_nodes, ncores, win):
    """Relabel nodes so each (core, window) bin carries a near-equal edge
    count: perm[old_id] = new_id. Greedy largest-degree-first into the
    lightest non-full bin."""
    import heapq
    deg = np.bincount(dst, minlength=n_nodes).astype(np.int64)
    nc_nodes = n_nodes // ncores
    nwin = (nc_nodes + win - 1) // win
    base = []
    cap = []
    for c in range(ncores):
        for w in range(nwin):
            base.append(c * nc_nodes + w * win)
            cap.append(min(win, nc_nodes - w * win))
    nbins = len(base)
    order = np.argsort(-deg, kind="stable")
    heap = [(0, b) for b in range(nbins)]
    heapq.heapify(heap)
    slot = [0] * nbins
    perm = np.zeros(n_nodes, dtype=np.int64)
    for node in order:
        while True:
            load, b = heapq.heappop(heap)
            if slot[b] < cap[b]:
                break
        perm[node] = base[b] + slot[b]
        slot[b] += 1
        if slot[b] < cap[b]:
            heapq.heappush(heap, (load + deg[node], b))
    return perm


def chunk_layout(n_nodes, ncores, chunk_tiles):
    """Chunked-AllGather table layout. Returns (bounds, rowmap) where bounds
    are per-core local row boundaries of each chunk and rowmap[node] is the
    table row of a global node id under chunk-major ordering. Chunk 0 ends at
    chunk_tiles*P (the A/B gather split); the last chunk is kept small so the
    final AllGather on the critical path (last window -> B gathers) is
    short."""
    nc_nodes = n_nodes // ncores
    c0 = chunk_tiles * P
    c2 = ((nc_nodes - c0) // 8 + P - 1) // P * P  # small last chunk (~3 tiles)
    bounds = [(0, c0), (c0, nc_nodes - c2), (nc_nodes - c2, nc_nodes)]
    rowmap = np.zeros(n_nodes, dtype=np.int64)
    out_base = 0
    for (lo, hi) in bounds:
        s = hi - lo
        for c in range(ncores):
            nodes = np.arange(c * nc_nodes + lo, c * nc_nodes + hi)
            rowmap[nodes] = out_base + c * s + np.arange(s)
        out_base += ncores * s
    return bounds, rowmap


def build_edge_data(src, dst, rowmap, n_nodes, ncores, win):
    """Two-stream (A/B) edge layout with per-window segments padded to the
    max across cores, plus per-core gather indices and fp8 one-hots.

    Returns (layout, percore):
      layout: nwin, TAtot, TBtot, jtot, wininfo (per window: list of
              (stream, ring_off, ntiles, j0) subruns), gathers trigger map.
      percore[c]: idxg [128, (TAtot+TBtot)*8] i16, ohe/ohd [128, jtot, 128] f8.
    """
    nc_nodes = n_nodes // ncores
    nwin = (nc_nodes + win - 1) // win
    order = np.argsort(dst, kind="stable")
    s_src, s_dst = src[order], dst[order]
    core_of = s_dst // nc_nodes
    win_of = (s_dst % nc_nodes) // win
    r2 = rowmap[s_src]
    isB = (r2 >= SPLIT).astype(np.int64)

    counts = np.zeros((ncores, nwin, 2), dtype=np.int64)
    np.add.at(counts, (core_of, win_of, isB), 1)
    LA = counts[:, :, 0].max(axis=0)  # [nwin] padded segment lengths
    LB = counts[:, :, 1].max(axis=0)
    SA = np.concatenate([[0], np.cumsum(LA)])  # segment starts
    SB = np.concatenate([[0], np.cumsum(LB)])
    LAtot, LBtot = int(SA[-1]), int(SB[-1])
    TAtot = (LAtot + P - 1) // P
    TBtot = (LBtot + P - 1) // P

    # per-window touching tiles and one-hot slot bases
    wininfo = []   # per window: list of (stream, tile0, ntiles, j0)
    jtot = 0
    wtiles = []    # per window: [(stream, tile, j)] in order
    for w in range(nwin):
        entry = []
        tl = []
        for stream, (S, L) in enumerate(((SA, LA), (SB, LB))):
            if L[w] == 0:
                continue
            t0 = int(S[w]) // P
            t1 = (int(S[w]) + int(L[w]) - 1) // P
            entry.append((stream, t0, t1 - t0 + 1, jtot))
            for t in range(t0, t1 + 1):
                tl.append((stream, t, jtot + t - t0))
            jtot += t1 - t0 + 1
        wininfo.append(entry)
        wtiles.append(tl)

    # per-edge slot: rank within (core, win, stream) bucket (dst-stable order)
    gid = (core_of * nwin + win_of) * 2 + isB
    E = gid.shape[0]
    sizes = np.bincount(gid, minlength=ncores * nwin * 2)
    starts = np.concatenate([[0], np.cumsum(sizes)])[:-1]
    order2 = np.argsort(gid, kind="stable")
    rank = np.empty(E, dtype=np.int64)
    rank[order2] = np.arange(E) - starts[gid[order2]]

    spos = np.where(isB == 0, SA[win_of], SB[win_of]) + rank
    tile = spos // P
    lane = spos % P
    dloc = s_dst - (core_of * nc_nodes + win_of * win)
    colbase = np.where(isB == 0, 0, 8 * TAtot)
    col = colbase + 8 * tile + lane // 16
    part = lane % 16
    idxval = np.where(isB == 0, r2, r2 - SPLIT).astype(np.int16)

    # one-hot slot j for each edge: j0(window, stream) + tile - t0
    j0A = np.full(nwin, -1, np.int64)
    t0A = np.zeros(nwin, np.int64)
    j0B = np.full(nwin, -1, np.int64)
    t0B = np.zeros(nwin, np.int64)
    for w in range(nwin):
        for (stream, t0, n, j0) in wininfo[w]:
            if stream == 0:
                j0A[w], t0A[w] = j0, t0
            else:
                j0B[w], t0B[w] = j0, t0
    jslot = np.where(isB == 0, j0A[win_of] + tile - t0A[win_of],
                     j0B[win_of] + tile - t0B[win_of])

    percore = []
    for c in range(ncores):
        m = core_of == c
        idx16 = np.zeros((16, (TAtot + TBtot) * 8), dtype=np.int16)
        idx16[part[m], col[m]] = idxval[m]
        idxg = np.tile(idx16, (8, 1))
        ohe = np.zeros((P, jtot, P), dtype=FP8)
        ohe[lane[m], jslot[m], dloc[m]] = 1.0
        ohd = np.zeros((P, jtot, P), dtype=FP8)
        ohd[dloc[m], jslot[m], lane[m]] = 1.0
        percore.append(dict(idxg=idxg, ohe=ohe, ohd=ohd))

    layout = dict(nwin=nwin, TAtot=TAtot, TBtot=TBtot, jtot=jtot,
                  wininfo=wininfo, wtiles=wtiles)
    return layout, percore


def build_host_inputs(x, edge_index, batch, W1, att_src1, att_dst1, b1,
                      W2, att_src2, att_dst2, b2, Wfc, bfc,
                      n_nodes, n_graphs, ncores, win):
    src, dst = np.asarray(edge_index[0]), np.asarray(edge_index[1])
    nc_nodes = n_nodes // ncores
    nt = (nc_nodes + P - 1) // P
    ncpad = nt * P

    bounds, rowmap = chunk_layout(n_nodes, ncores, CHUNK_TILES)
    layout, edata = build_edge_data(src.astype(np.int64), dst, rowmap,
                                    n_nodes, ncores, win)

    in_c = W1.shape[0]
    A1 = np.zeros((in_c, 2 * HEADS), dtype=np.float64)
    for h in range(HEADS):
        A1[:, h] = W1[:, h * HID_C:(h + 1) * HID_C].astype(np.float64) @ att_src1[h].astype(np.float64)
        A1[:, HEADS + h] = W1[:, h * HID_C:(h + 1) * HID_C].astype(np.float64) @ att_dst1[h].astype(np.float64)
    W1aug = np.concatenate([W1.astype(np.float64), A1], axis=1).astype(BF16)  # [in_c, 264]

    hid2 = W2.shape[0]
    A2 = np.zeros((hid2, 2), dtype=np.float64)
    A2[:, 0] = W2.astype(np.float64) @ att_src2[0].astype(np.float64)
    A2[:, 1] = W2.astype(np.float64) @ att_dst2[0].astype(np.float64)
    W2aug = np.concatenate([W2.astype(np.float64), A2], axis=1).astype(BF16)  # [hid2, 258]

    cnt = np.bincount(batch, minlength=n_graphs).astype(np.float32)
    cnt_inv = (1.0 / np.maximum(cnt, 1.0)).astype(np.float32)

    common = dict(
        w1aug=np.ascontiguousarray(W1aug),
        w2aug=np.ascontiguousarray(W2aug),
        wfc=np.ascontiguousarray(Wfc.astype(BF16)),
        b1rep=np.ascontiguousarray(np.broadcast_to(b1.astype(np.float32), (win, b1.shape[0])).copy()),
        b2rep=np.ascontiguousarray(np.broadcast_to(b2.astype(np.float32), (win, b2.shape[0])).copy()),
        bfc2=np.ascontiguousarray(bfc.astype(np.float32).reshape(2, P).T.copy()),
        cinv=np.ascontiguousarray(np.broadcast_to(cnt_inv, (P, n_graphs)).copy()),
    )

    per_core = []
    for c in range(ncores):
        xt = np.zeros((in_c, ncpad), dtype=BF16)
        xs = x[c * nc_nodes:(c + 1) * nc_nodes].astype(np.float32)
        xt[:, :nc_nodes] = np.ascontiguousarray(xs.T).astype(BF16)
        gmap = np.zeros((P, nt, n_graphs), dtype=np.float32)
        nodes = np.arange(nc_nodes)
        gmap[nodes % P, nodes // P, batch[c * nc_nodes:(c + 1) * nc_nodes]] = 1.0
        d = edata[c]
        per_core.append(dict(
            xt=xt,
            idxg=np.ascontiguousarray(d["idxg"]),
            ohe=np.ascontiguousarray(d["ohe"]),
            ohd=np.ascontiguousarray(d["ohd"]),
            gmap=np.ascontiguousarray(gmap.astype(BF16)),
            **common,
        ))
    return layout, per_core


# --------------------------------------------------------------------------
# device program
# --------------------------------------------------------------------------

def build_program(layout, n_nodes, n_graphs, ncores, win,
                  dma_scratch=40960):
    bounds, _ = chunk_layout(n_nodes, ncores, CHUNK_TILES)
    from concourse import bass, bacc, mybir, tile, library_config
    from concourse.masks import make_identity

    DT = mybir.dt.bfloat16
    F8 = mybir.dt.float8e4
    F32 = mybir.dt.float32
    I16 = mybir.dt.int16
    AF = mybir.ActivationFunctionType
    OP = mybir.AluOpType

    nc_nodes = n_nodes // ncores
    nt = (nc_nodes + P - 1) // P
    nwin = layout["nwin"]
    TAtot, TBtot, jtot = layout["TAtot"], layout["TBtot"], layout["jtot"]
    wininfo = layout["wininfo"]
    JWMAX = max(sum(n for _, _, n, _ in e) for e in wininfo)
    in_c, out_c = IN_C, OUT_C
    G = n_graphs

    nc = bacc.Bacc("TRN2", target_bir_lowering=False, num_devices=ncores,
                   dynamic_dma_scratch_size=dma_scratch, num_swdge_queues=NQ)

    # ---- dram i/o ----
    xt_d = nc.dram_tensor("xt", [in_c, nt * P], DT, kind="ExternalInput")
    w1_d = nc.dram_tensor("w1aug", [in_c, 264], DT, kind="ExternalInput")
    w2_d = nc.dram_tensor("w2aug", [in_c, 258], DT, kind="ExternalInput")
    wfc_d = nc.dram_tensor("wfc", [out_c, out_c], DT, kind="ExternalInput")
    idxg_d = nc.dram_tensor("idxg", [P, (TAtot + TBtot) * 8], I16, kind="ExternalInput")
    ohe_d = nc.dram_tensor("ohe", [P, jtot, P], F8, kind="ExternalInput")
    ohd_d = nc.dram_tensor("ohd", [P, jtot, P], F8, kind="ExternalInput")
    gmap_d = nc.dram_tensor("gmap", [P, nt, G], DT, kind="ExternalInput")
    b1_d = nc.dram_tensor("b1rep", [win, out_c], F32, kind="ExternalInput")
    b2_d = nc.dram_tensor("b2rep", [win, out_c], F32, kind="ExternalInput")
    bfc_d = nc.dram_tensor("bfc2", [P, 2], F32, kind="ExternalInput")
    cinv_d = nc.dram_tensor("cinv", [P, G], F32, kind="ExternalInput")
    y_d = nc.dram_tensor("y", [out_c, G], F32, kind="ExternalOutput")

    cin1 = nc.dram_tensor("cin1", [nc_nodes, TBW], DT, kind="Internal")
    tab1 = nc.dram_tensor("tab1", [n_nodes, TBW], DT, kind="Internal", addr_space="Shared")
    cin2 = nc.dram_tensor("cin2", [nc_nodes, TBW], DT, kind="Internal")
    tab2 = nc.dram_tensor("tab2", [n_nodes, TBW], DT, kind="Internal", addr_space="Shared")
    pin = nc.dram_tensor("pin", [out_c, G], F32, kind="Internal")
    pout = nc.dram_tensor("pout", [out_c, G], F32, kind="Internal", addr_space="Shared")

    groups_rg = [list(range(ncores))]

    with tile.TileContext(nc) as tc:
        with (
            tc.tile_pool(name="const", bufs=1) as cpool,
            tc.tile_pool(name="xs", bufs=2) as xpool,
            tc.tile_pool(name="work", bufs=3) as wpool,
            tc.tile_pool(name="rhsp", bufs=2) as rpool,
            tc.tile_pool(name="oh", bufs=3) as opool,
            tc.tile_pool(name="ring", bufs=1) as ringpool,
            tc.tile_pool(name="np", bufs=2, space="PSUM") as npp,
            tc.tile_pool(name="agg", bufs=2, space="PSUM") as aggp,
            tc.tile_pool(name="adp", bufs=1, space="PSUM") as adp,
            tc.tile_pool(name="trp", bufs=1, space="PSUM") as trp,
            tc.tile_pool(name="plp", bufs=1, space="PSUM") as plp,
        ):
            # ---- constants ----
            ident = cpool.tile([P, P], DT)
            make_identity(nc, ident[:])
            nc.gpsimd.load_library(library_config.mlp)
            w1_sb = cpool.tile([P, 2, 264], DT)
            nc.sync.dma_start(out=w1_sb[:, :, :], in_=w1_d.ap().rearrange("(kh p) m -> p kh m", p=P))
            w2_sb = cpool.tile([P, 2, 258], DT)
            nc.sync.dma_start(out=w2_sb[:, :, :], in_=w2_d.ap().rearrange("(kh p) m -> p kh m", p=P))
            wfc_sb = cpool.tile([P, 2, 2, P], DT)
            nc.sync.dma_start(out=wfc_sb[:, :, :, :],
                              in_=wfc_d.ap().rearrange("(kh p) (mh q) -> p kh mh q", p=P, q=P))
            b1_sb = cpool.tile([win, out_c], F32)
            nc.sync.dma_start(out=b1_sb[:, :], in_=b1_d[:, :])
            b2_sb = cpool.tile([win, out_c], F32)
            nc.sync.dma_start(out=b2_sb[:, :], in_=b2_d[:, :])
            bfc_sb = cpool.tile([P, 2], F32)
            nc.sync.dma_start(out=bfc_sb[:, :], in_=bfc_d[:, :])
            cinv_sb = cpool.tile([P, G], F32)
            nc.sync.dma_start(out=cinv_sb[:, :], in_=cinv_d[:, :])
            ixg_sb = cpool.tile([P, (TAtot + TBtot) * 8], I16)
            nc.sync.dma_start(out=ixg_sb[:, :], in_=idxg_d[:, :])
            gmap_sb = cpool.tile([P, nt, G], DT)
            nc.sync.dma_start(out=gmap_sb[:, :, :], in_=gmap_d[:, :, :])
            adc1_sb = cpool.tile([P, nt, HEADS], DT)
            adc2_sb = cpool.tile([P, nt, HEADS], DT)

            # ---- node phase ----
            chunk_end_tile = {}
            chunk_ob = []
            ob = 0
            for k, (lo, hi) in enumerate(bounds):
                chunk_end_tile[(hi + P - 1) // P - 1] = k
                chunk_ob.append(ob)
                ob += ncores * (hi - lo)

            def ag_chunk(cin, tab, k):
                lo, hi = bounds[k]
                s = hi - lo
                nc.gpsimd.collective_compute(
                    "AllGather", mybir.AluOpType.bypass,
                    ins=[cin.ap()[lo:hi, :]],
                    outs=[tab.ap()[chunk_ob[k]:chunk_ob[k] + ncores * s, :]],
                    replica_groups=groups_rg)

            def node_tile(t, lhsT_of, w_sb, ocols, cin, adc_sb, H):
                rows = min(P, nc_nodes - t * P)
                ps = npp.tile([P, 264], F32, tag="nps", name="nps")
                for kh in range(2):
                    nc.tensor.matmul(out=ps[:rows, :ocols], lhsT=lhsT_of(t, kh, rows),
                                     rhs=w_sb[:, kh, :ocols], start=(kh == 0), stop=(kh == 1))
                sb = wpool.tile([P, TBW], DT, tag="nsb", name="nsb")
                nc.vector.tensor_copy(out=sb[:rows, 0:ACOL].bitcast(F8),
                                      in_=ps[:rows, 0:PCOL])
                nc.vector.tensor_copy(out=sb[:rows, ACOL:ACOL + H],
                                      in_=ps[:rows, PCOL:PCOL + H])
                nc.vector.tensor_copy(out=adc_sb[:rows, t, :H],
                                      in_=ps[:rows, PCOL + H:PCOL + 2 * H])
                nc.sync.dma_start(out=cin.ap()[t * P:t * P + rows, :], in_=sb[:rows, :])

            XCH = 5
            def l1_lhsT_factory():
                state = {}
                def get(t, kh, rows):
                    c0 = (t // XCH) * XCH
                    if state.get("c0") != c0:
                        xt_sb = xpool.tile([P, 2, XCH * P], DT, tag="xt")
                        hi = min(c0 * P + XCH * P, nt * P)
                        for k2 in range(2):
                            nc.sync.dma_start(out=xt_sb[:, k2, :hi - c0 * P],
                                              in_=xt_d[k2 * P:(k2 + 1) * P, c0 * P:hi])
                        state["c0"] = c0
                        state["sb"] = xt_sb
                    return state["sb"][:, kh, (t - c0) * P:(t - c0) * P + rows]
                return get

            l1_lhsT = l1_lhsT_factory()
            for t in range(nt):
                node_tile(t, l1_lhsT, w1_sb, 264, cin1, adc1_sb, HEADS)
                if t in chunk_end_tile:
                    ag_chunk(cin1, tab1, chunk_end_tile[t])

            # ---- edge phase ----
            qct = [0]

            def edge_phase(tab, cin, brep, H, adc_sb, li, pool_into=None, after_window=None):
                RH = H + out_c
                RING, PF = RINGL[li], PFL[li]
                rings = [ringpool.tile([P, RING, TBW], DT, tag=f"ringA{li}", name="ringA"),
                         ringpool.tile([P, RING, TBW], DT, tag=f"ringB{li}", name="ringB")]
                Ttot = (TAtot, TBtot)
                cbase = (0, 8 * TAtot)
                sbase = (0, SPLIT)
                send = (SPLIT, n_nodes)
                issued = [0, 0]  # tiles issued per stream

                def issue(stream, upto):
                    while issued[stream] < min(upto, Ttot[stream]):
                        t0 = issued[stream]
                        ntl = min(GMAX, Ttot[stream] - t0)
                        rs = t0 % RING
                        nc.gpsimd.dma_gather(
                            out_ap=rings[stream][:, rs:rs + ntl, :],
                            in_ap=tab.ap()[sbase[stream]:send[stream], :],
                            idxs_ap=ixg_sb[:, cbase[stream] + 8 * t0:
                                           cbase[stream] + 8 * (t0 + ntl)],
                            num_idxs=P * ntl, num_idxs_reg=P * ntl,
                            elem_size=TBW, queue_num=qct[0] % NQ)
                        qct[0] += 1
                        issued[stream] += ntl

                for w in range(nwin):
                    size = min(win, nc_nodes - w * win)
                    entry = wininfo[w]
                    # subruns: split window tile ranges at ring wrap
                    subs = []  # (stream, ring_off, n)
                    jw = 0
                    for (stream, t0, n, j0) in entry:
                        issue(stream, t0 + n + PF)
                        t = t0
                        while t < t0 + n:
                            rs = t % RING
                            nrun = min(t0 + n - t, RING - rs)
                            subs.append((stream, rs, nrun))
                            t += nrun
                        jw += n
                    jbase = entry[0][3]
                    # one-hots for this window
                    oe = opool.tile([P, JWMAX, P], F8, tag="oe")
                    nc.sync.dma_start(out=oe[:, :jw, :], in_=ohe_d[:, jbase:jbase + jw, :])
                    od = opool.tile([P, JWMAX, P], F8, tag="od")
                    nc.sync.dma_start(out=od[:, :jw, :], in_=ohd_d[:, jbase:jbase + jw, :])
                    # own-dst attention coefs (stashed in SBUF by the node phase)
                    ad = adc_sb[:, w, :]
                    # a_dst expanded to edges
                    adps = adp.tile([P, (JWMAX + 1) * HEADS], F32, tag="adps")
                    j = 0
                    for (stream, rs, n) in subs:
                        for t in range(n):
                            nc.tensor.matmul(out=adps[:, (j + t) * H:(j + t + 1) * H],
                                             lhsT=od[:size, j + t, :], rhs=ad[:size, :H],
                                             start=True, stop=True)
                        j += n
                    # logits -> exp(leaky_relu)
                    lg = wpool.tile([P, (JWMAX + 1) * HEADS], F32, tag="lg")
                    j = 0
                    for (stream, rs, n) in subs:
                        nc.vector.tensor_tensor(
                            out=lg[:, j * H:(j + n) * H].rearrange("p (t h) -> p t h", t=n),
                            in0=rings[stream][:, rs:rs + n, ACOL:ACOL + H],
                            in1=adps[:, j * H:(j + n) * H].rearrange("p (t h) -> p t h", t=n),
                            op=OP.add)
                        j += n
                    lk = wpool.tile([P, (JWMAX + 1) * HEADS], F32, tag="lk")
                    nc.scalar.activation(out=lk[:, :jw * H], in_=lg[:, :jw * H],
                                         func=AF.Copy, scale=NEG_SLOPE)
                    nc.vector.tensor_tensor(out=lk[:, :jw * H], in0=lg[:, :jw * H],
                                            in1=lk[:, :jw * H], op=OP.max)
                    rhs = rpool.tile([P, JWMAX, HEADS + out_c], DT, tag="rhs")
                    nc.scalar.activation(out=rhs[:, :jw, 0:H],
                                         in_=lk[:, :jw * H].rearrange("p (t h) -> p t h", t=jw),
                                         func=AF.Exp)
                    C2 = out_c // H
                    j = 0
                    for (stream, rs, n) in subs:
                        nc.vector.tensor_tensor(
                            out=rhs[:, j:j + n, H:RH].rearrange("p t (h c) -> p t h c", h=H),
                            in0=rings[stream][:, rs:rs + n, 0:ACOL].bitcast(F8)
                                .rearrange("p t (h c) -> p t h c", h=H),
                            in1=rhs[:, j:j + n, 0:H].to_broadcast([P, n, H, C2]),
                            op=OP.mult)
                        j += n
                    # aggregate into [win, RH]
                    ag = aggp.tile([win, HEADS + out_c], F32, tag="ag")
                    for j in range(jw):
                        nc.tensor.matmul(out=ag[:, :RH], lhsT=oe[:, j, :],
                                         rhs=rhs[:, j, :RH],
                                         start=(j == 0), stop=(j == jw - 1))
                    # normalize + bias + relu
                    s = wpool.tile([win, HEADS], F32, tag="s")
                    nc.vector.tensor_scalar_max(s[:size, :H], ag[:size, 0:H], 1e-30)
                    nc.vector.reciprocal(out=s[:size, :H], in_=s[:size, :H])
                    on = wpool.tile([win, out_c], F32, tag="on")
                    for h in range(H):
                        nc.scalar.activation(out=on[:size, h * C2:(h + 1) * C2],
                                             in_=ag[:size, H + h * C2:H + (h + 1) * C2],
                                             func=AF.Copy, scale=s[:size, h:h + 1])
                    nc.vector.tensor_tensor(out=on[:size, :], in0=on[:size, :],
                                            in1=brep[:size, :], op=OP.add)
                    ro = wpool.tile([win, out_c], DT, tag="ro")
                    nc.scalar.activation(out=ro[:size, :], in_=on[:size, :], func=AF.Relu)
                    if after_window is not None:
                        after_window(w, ro, size)
                    if pool_into is not None:
                        for mh in range(2):
                            nc.tensor.matmul(out=pool_into[mh][:, :],
                                             lhsT=ro[:size, mh * P:(mh + 1) * P],
                                             rhs=gmap_sb[:size, w, :],
                                             start=(w == 0), stop=(w == nwin - 1))

            def l1_after_window(w, ro, size):
                def l2_lhsT(t, kh, rows):
                    tp = trp.tile([P, P], DT, tag="tp", name="tp")
                    nc.tensor.transpose(out=tp[:, :rows], in_=ro[:rows, kh * P:(kh + 1) * P],
                                        identity=ident[:rows, :rows])
                    tl = wpool.tile([P, P], DT, tag="tl", name="tl")
                    nc.vector.tensor_copy(out=tl[:, :rows], in_=tp[:, :rows])
                    return tl[:, :rows]
                node_tile(w, l2_lhsT, w2_sb, 258, cin2, adc2_sb, 1)
                if w in chunk_end_tile:
                    ag_chunk(cin2, tab2, chunk_end_tile[w])

            edge_phase(tab1, cin1, b1_sb, HEADS, adc1_sb, 1, after_window=l1_after_window)

            assert win == P and nwin == nt
            pps = [plp.tile([P, G], F32, tag=f"pp{mh}", name=f"pp{mh}") for mh in range(2)]
            edge_phase(tab2, cin2, b2_sb, 1, adc2_sb, 2, pool_into=pps)

            # ---- pool + fc ----
            psb = wpool.tile([P, 2, G], F32, tag="psb")
            for mh in range(2):
                nc.vector.tensor_copy(out=psb[:, mh, :], in_=pps[mh][:, :])
            nc.sync.dma_start(out=pin.ap().rearrange("(mh p) g -> p mh g", p=P), in_=psb[:, :, :])

            nc.gpsimd.collective_compute(
                "AllReduce", mybir.AluOpType.add,
                ins=[pin.ap()], outs=[pout.ap()], replica_groups=groups_rg)

            pr = wpool.tile([P, 2, G], F32, tag="pr")
            nc.sync.dma_start(out=pr[:, :, :], in_=pout.ap().rearrange("(mh p) g -> p mh g", p=P))
            pm = wpool.tile([P, 2, G], DT, tag="pm")
            for kh in range(2):
                nc.vector.tensor_tensor(out=pm[:, kh, :], in0=pr[:, kh, :], in1=cinv_sb[:, :], op=OP.mult)
            for mh in range(2):
                fps = aggp.tile([P, G], F32, tag="ag")
                for kh in range(2):
                    nc.tensor.matmul(out=fps[:, :], lhsT=wfc_sb[:, kh, mh, :], rhs=pm[:, kh, :],
                                     start=(kh == 0), stop=(kh == 1))
                yo = wpool.tile([P, G], F32, tag="yo")
                nc.scalar.activation(out=yo[:, :], in_=fps[:, :], func=AF.Relu,
                                     bias=bfc_sb[:, mh:mh + 1], scale=1.0)
                nc.sync.dma_start(out=y_d[mh * P:(mh + 1) * P, :], in_=yo[:, :])

    nc.compile()
    return nc


def _install_ntff_hook():
    """Register the NTFF profile hook (the image's antenv lacks axon_hooks)."""
    import types
    mod = sys.modules.get("antenv.axon_hooks")
    if mod is None:
        import antenv
        mod = types.ModuleType("antenv.axon_hooks")
        mod._hook = None
        mod.set_axon_ntff_profile_hook = lambda h: setattr(mod, "_hook", h)
        mod.get_axon_ntff_profile_hook = lambda: mod._hook
        sys.modules["antenv.axon_hooks"] = mod
        antenv.axon_hooks = mod
    if mod._hook is None:
        from trn_agent_boot.trn_boot import _ntff_profile_via_ctypes
        mod.set_axon_ntff_profile_hook(_ntff_profile_via_ctypes("/opt/axon/libaxon_pjrt.so"))


# --------------------------------------------------------------------------
# entry point
# --------------------------------------------------------------------------

def kernel(**inputs) -> np.ndarray:
    global LAST_EXEC_NS
    from concourse.bass_utils import run_bass_kernel_spmd

    args = {k: np.asarray(v) for k, v in inputs.items()}
    perm = balance_nodes(args["edge_index"][1], N_NODES, NCORES, WIN)
    old_of_new = np.argsort(perm)
    args["x"] = args["x"][old_of_new]
    args["batch"] = args["batch"][old_of_new]
    ei = args["edge_index"]
    args["edge_index"] = np.stack([perm[ei[0]], perm[ei[1]]]).astype(ei.dtype)
    layout, per_core = build_host_inputs(
        args["x"], args["edge_index"], args["batch"],
        args["W1"], args["att_src1"], args["att_dst1"], args["b1"],
        args["W2"], args["att_src2"], args["att_dst2"], args["b2"],
        args["Wfc"], args["bfc"],
        N_NODES, N_GRAPHS, NCORES, WIN)
    nc = build_program(layout, N_NODES, N_GRAPHS, NCORES, WIN)

    trace = os.environ.get("GAT_TRACE") == "1"
    if trace:
        try:
            _install_ntff_hook()
        except Exception:
            trace = False
    res = run_bass_kernel_spmd(nc, per_core, core_ids=list(range(NCORES)), trace=trace)
    LAST_EXEC_NS = res.exec_time_ns
    y = res.results[0]["y"]
    return np.ascontiguousarray(y.T).astype(np.float32)
